# revision 33
# baseline (speedup 1.0000x reference)
"""Trainium2 Bass kernel for nn_BDHLayer (sparse attention / BDH layer).

Sharding: 16 heads across 8 cores (2 heads per core, tensor parallel).
Decoder partial sums are combined with an on-chip ReduceScatter; each core
then applies the final layernorm+residual+rmsnorm to its T/8 slice.

All matmuls run in bf16 (fp32 PSUM accumulation). Host pre-transposes
weights/activations so every contraction dim lands on SBUF partitions.
The middle layernorm is applied as a post-GEMM correction:
  sqrelu(LN(yKV) @ Wv^T) = min(v,0)*v*r^2 with v = Wsum*mu - z,
  z = yKV @ Wv^T, Wsum = sum_d Wv, r^2 = 1/(var+eps).
"""

import sys

sys.path.insert(0, '/opt/trn_rl_repo')

import numpy as np
import ml_dtypes

import concourse.bass as bass
import concourse.bacc as bacc
import concourse.mybir as mybir
from concourse import tile
from concourse import bass_utils

BF = ml_dtypes.bfloat16
FP32 = np.float32

B, T, D = 1, 2048, 1024
NH = 16
N = 1024            # neurons per head
CS = 256            # rotary chunk size
BASE = 2.0 ** 16
SCALE_BASE = 512.0
LN_EPS = 1e-5
RMS_EPS = 1e-6

NCORES = 8
HPC = NH // NCORES  # heads per core = 2
TS = T // NCORES    # output rows per core = 256

NT = N // 128       # 8 n-tiles per head
DT = D // 128       # 8 d-tiles
TT = T // 128       # 16 t-tiles
TB = T // 512       # 4 t-blocks
DB = D // 512       # 2 d-blocks

dt = mybir.dt
Alu = mybir.AluOpType
Act = mybir.ActivationFunctionType


# ---------------------------------------------------------------- host tables

def _rope_tables():
    idx = np.arange(0, CS, 2, dtype=np.float64)
    inv_freq = 1.0 / (BASE ** (idx / CS))
    t = np.arange(T, dtype=np.float64)
    freqs = t[:, None] * inv_freq[None, :]              # (T, 128)
    scale_vec = (idx + 0.4 * CS) / (1.4 * CS)
    power = (t - T // 2) / SCALE_BASE
    scale = scale_vec[None, :] ** power[:, None]        # (T, 128)
    cos = (np.cos(freqs) * scale).astype(np.float32)
    sin = (np.sin(freqs) * scale).astype(np.float32)
    # transpose to (128, T): row = pair index within chunk, col = t
    return np.ascontiguousarray(cos.T), np.ascontiguousarray(sin.T)


def _masks():
    # scoresT tile layout: [u_p (128), t_f (512)]; diagonal block j keeps
    # strictly-causal u < t, i.e. 128*j + u_p < t_f.
    m = np.zeros((4, 128, 512), dtype=np.float32)
    up = np.arange(128)[:, None]
    tf = np.arange(512)[None, :]
    for j in range(4):
        m[j] = (128 * j + up < tf).astype(np.float32)
    return m


# ------------------------------------------------------------------- builder

def _emit(nc, tc, tens, debug=False):
    x_bf, xT_bf, xs_f32 = tens['x_bf'], tens['xT_bf'], tens['xs_f32']
    wencT, wencvT, wdecT = tens['wencT'], tens['wencvT'], tens['wdecT']
    wsumT, cosT_d, sinT_d, masks_d = (tens['wsumT'], tens['cosT'],
                                      tens['sinT'], tens['masks'])
    out_d, xy_d = tens['out'], tens['xy_d']
    bounce_in, bounce_out = tens['bounce_in'], tens['bounce_out']

    f32, bf16 = dt.float32, dt.bfloat16

    from contextlib import ExitStack
    with ExitStack() as ctx:
        p_const = ctx.enter_context(
            tc.tile_pool(name="const", bufs=1, side="right"))
        p_x = ctx.enter_context(
            tc.tile_pool(name="xnat", bufs=1, side="right"))
        p_psum = ctx.enter_context(
            tc.tile_pool(name="psum", bufs=7, space="PSUM"))
        p_psum_v = ctx.enter_context(
            tc.tile_pool(name="psumv", bufs=1, space="PSUM"))

        # ---- constants
        cos_sb = p_const.tile([128, T], bf16, tag="cos")
        sin_sb = p_const.tile([128, T], bf16, tag="sin")
        nc.gpsimd.dma_start(cos_sb[:], cosT_d[:])
        nc.gpsimd.dma_start(sin_sb[:], sinT_d[:])
        mask_sb = p_const.tile([128, 4 * 512], f32, tag="masks")
        wsum_sb = p_const.tile([128, HPC * NT], f32, tag="wsum")
        for h in range(HPC):
            nc.sync.dma_start(wsum_sb[:, h * NT:(h + 1) * NT],
                              wsumT[h, :, :])
        ones_bf = p_const.tile([128, 1], bf16, tag="ones_bf")
        nc.vector.memset(ones_bf[:], 1.0)
        ones_row = p_const.tile([1, 128], bf16, tag="ones_row")
        nc.vector.memset(ones_row[:], 1.0)

        # ---- x natural layout (persistent): tile tt holds x[128tt:+128, :]
        x_sb = p_x.tile([128, TT * D], bf16, tag="x")
        bulk_loaded = [False]

        def load_bulk():
            if bulk_loaded[0]:
                return
            bulk_loaded[0] = True
            for j in range(4):
                nc.gpsimd.dma_start(mask_sb[:, j * 512:(j + 1) * 512],
                                    masks_d[j, :, :])
            for tt in range(TT):
                nc.gpsimd.dma_start(x_sb[:, tt * D:(tt + 1) * D],
                                    x_bf[tt * 128:(tt + 1) * 128, :])

        for h in range(HPC):
            with ExitStack() as hctx:
                # x_sparse^2 (pre-rope), full head — lives enc -> gating
                p_qsq = hctx.enter_context(
                    tc.tile_pool(name=f"qsq{h}", bufs=1, side="right"))
                qsq = p_qsq.tile([128, NT * T], bf16, tag="qsq")

                # =========================== ENC + ROPE ====================
                with ExitStack() as ectx:
                    p_qtr = ectx.enter_context(
                        tc.tile_pool(name=f"qtr{h}", bufs=1))
                    qtr = p_qtr.tile([128, NT * T], bf16, tag="qtr")

                    with ExitStack() as e2:
                        p_enc = e2.enter_context(
                            tc.tile_pool(name=f"enc{h}", bufs=1))
                        p_xts = e2.enter_context(
                            tc.tile_pool(name=f"xts{h}", bufs=1))
                        p_scr = e2.enter_context(
                            tc.tile_pool(name=f"escr{h}", bufs=3))

                        wenc_sb = p_enc.tile([128, DT * N], bf16, tag="wenc")
                        for dtt in range(DT):
                            eng = nc.sync if dtt % 2 == 0 else nc.gpsimd
                            eng.dma_start(
                                wenc_sb[:, dtt * N:(dtt + 1) * N],
                                wencT[h, dtt * 128:(dtt + 1) * 128, :])

                        # full xT resident; slices arrive per (tb, dtt)
                        xfull = p_xts.tile([128, TB * DT * 512], bf16,
                                           tag="xts")
                        for tb in range(TB):
                            for dtt in range(DT):
                                eng = nc.sync if dtt % 2 == 1 else nc.gpsimd
                                eng.dma_start(
                                    xfull[:, tb * DT * 512 + dtt * 512:
                                          tb * DT * 512 + (dtt + 1) * 512],
                                    xT_bf[dtt * 128:(dtt + 1) * 128,
                                          tb * 512:(tb + 1) * 512])
                        p_rt = e2.enter_context(
                            tc.tile_pool(name=f"rt{h}", bufs=3))
                        for nt in range(NT):
                            for tb in range(TB):
                                ps = p_psum.tile([128, 512], f32, tag="mm")
                                for dtt in range(DT):
                                    nc.tensor.matmul(
                                        ps[:],
                                        wenc_sb[:, dtt * N + nt * 128:
                                                dtt * N + nt * 128 + 128],
                                        xfull[:, tb * DT * 512 + dtt * 512:
                                              tb * DT * 512 + (dtt + 1) * 512],
                                        start=(dtt == 0), stop=(dtt == DT - 1))
                                relu_t = p_scr.tile([128, 512], f32,
                                                    tag="relu")
                                nc.scalar.activation(relu_t[:], ps[:],
                                                     Act.Relu)
                                nc.vector.tensor_mul(
                                    qsq[:, nt * T + tb * 512:
                                        nt * T + tb * 512 + 512],
                                    relu_t[:], relu_t[:])
                            if nt % 2 == 1:
                                # rope chunk (nt-1, nt) as soon as pair done
                                c = nt // 2
                                a = qsq[:, (2 * c) * T:(2 * c + 1) * T]
                                b = qsq[:, (2 * c + 1) * T:(2 * c + 2) * T]
                                t1 = p_rt.tile([128, T], bf16, tag="rtmp")
                                t2 = p_rt.tile([128, T], bf16, tag="rtmp")
                                nc.vector.tensor_mul(t1[:], a, cos_sb[:])
                                nc.vector.tensor_mul(t2[:], b, sin_sb[:])
                                nc.vector.tensor_sub(
                                    qtr[:, (2 * c) * T:(2 * c + 1) * T],
                                    t1[:], t2[:])
                                t3 = p_rt.tile([128, T], bf16, tag="rtmp")
                                t4 = p_rt.tile([128, T], bf16, tag="rtmp")
                                nc.vector.tensor_mul(t3[:], b, cos_sb[:])
                                nc.vector.tensor_mul(t4[:], a, sin_sb[:])
                                nc.vector.tensor_add(
                                    qtr[:, (2 * c + 1) * T:(2 * c + 2) * T],
                                    t3[:], t4[:])

                    load_bulk()
                    if debug and h == 0:
                        nc.sync.dma_start(tens['dbg_qtr'][:, :], qtr[:])

                    # ======================= SCORES + yKV ===================
                    p_ykv = hctx.enter_context(
                        tc.tile_pool(name=f"ykv{h}", bufs=1, side="right"))
                    p_vec = hctx.enter_context(
                        tc.tile_pool(name=f"vec{h}", bufs=1, side="right"))
                    p_vrow = hctx.enter_context(
                        tc.tile_pool(name=f"vrow{h}", bufs=1, side="right"))
                    ykv = p_ykv.tile([128, DT * T], bf16, tag="ykv")
                    # stat rows broadcast across partitions (via K=1 matmul)
                    mu_b = p_vec.tile([128, T], f32, tag="mu_b")
                    r2_b = p_vec.tile([128, T], bf16, tag="r2_b")

                    with ExitStack() as sctx:
                        p_sct = sctx.enter_context(
                            tc.tile_pool(name=f"sct{h}", bufs=1))
                        p_sq = sctx.enter_context(
                            tc.tile_pool(name=f"sq{h}", bufs=1))

                        for tb in range(TB):
                            ub_max = 4 * tb + 4
                            sct = p_sct.tile([128, 16 * 512], bf16, tag="sct")
                            for ub in range(ub_max):
                                j = ub - 4 * tb
                                off = 128 * j if j > 0 else 0
                                w = 512 - off
                                ps = p_psum.tile([128, 512], f32, tag="mm")
                                for nt in range(NT):
                                    nc.tensor.matmul(
                                        ps[:, :w],
                                        qtr[:, nt * T + ub * 128:
                                            nt * T + ub * 128 + 128],
                                        qtr[:, nt * T + tb * 512 + off:
                                            nt * T + (tb + 1) * 512],
                                        start=(nt == 0), stop=(nt == NT - 1))
                                base = ub * 512
                                if j >= 0:
                                    if off:
                                        nc.vector.memset(
                                            sct[:, base:base + off], 0.0)
                                    nc.vector.tensor_mul(
                                        sct[:, base + off:base + 512],
                                        ps[:, :w],
                                        mask_sb[:, j * 512 + off:
                                                (j + 1) * 512])
                                else:
                                    nc.scalar.copy(sct[:, base:base + 512],
                                                   ps[:])

                            sq_full = p_sq.tile([128, DT * 512], bf16,
                                                tag="sq")
                            for dtt in range(DT):
                                ps2 = p_psum.tile([128, 512], f32, tag="mm")
                                for ub in range(ub_max):
                                    nc.tensor.matmul(
                                        ps2[:],
                                        x_sb[:, ub * D + dtt * 128:
                                             ub * D + dtt * 128 + 128],
                                        sct[:, ub * 512:(ub + 1) * 512],
                                        start=(ub == 0),
                                        stop=(ub == ub_max - 1))
                                nc.scalar.copy(
                                    ykv[:, dtt * T + tb * 512:
                                        dtt * T + tb * 512 + 512], ps2[:])
                                nc.scalar.square(
                                    sq_full[:, dtt * 512:(dtt + 1) * 512],
                                    ps2[:])

                            mean_ps = p_psum_v.tile([1, 512], f32, tag="st")
                            for dtt in range(DT):
                                nc.tensor.matmul(
                                    mean_ps[:], ones_bf[:],
                                    ykv[:, dtt * T + tb * 512:
                                        dtt * T + tb * 512 + 512],
                                    start=(dtt == 0), stop=(dtt == DT - 1))
                            ssq_ps = p_psum_v.tile([1, 512], f32, tag="st")
                            for dtt in range(DT):
                                nc.tensor.matmul(
                                    ssq_ps[:], ones_bf[:],
                                    sq_full[:, dtt * 512:(dtt + 1) * 512],
                                    start=(dtt == 0), stop=(dtt == DT - 1))
                            sl = slice(tb * 512, (tb + 1) * 512)
                            mu_r = p_vrow.tile([1, 512], bf16, tag="mu_r")
                            ssq_r = p_vrow.tile([1, 512], f32, tag="ssq_r")
                            musq_r = p_vrow.tile([1, 512], f32, tag="musq_r")
                            r2_r = p_vrow.tile([1, 512], bf16, tag="r2_r")
                            nc.scalar.mul(mu_r[:], mean_ps[:], 1.0 / D)
                            nc.scalar.mul(ssq_r[:], ssq_ps[:], 1.0 / D)
                            nc.vector.tensor_mul(musq_r[:], mu_r[:], mu_r[:])
                            nc.vector.tensor_sub(ssq_r[:], ssq_r[:],
                                                 musq_r[:])
                            nc.vector.tensor_scalar_add(
                                ssq_r[:], ssq_r[:], LN_EPS)
                            nc.vector.reciprocal(ssq_r[:], ssq_r[:])
                            nc.vector.tensor_copy(r2_r[:], ssq_r[:])
                            # broadcast mu/r2 rows to all 128 partitions
                            bc1 = p_psum_v.tile([128, 512], f32, tag="st")
                            nc.tensor.matmul(bc1[:], ones_row[:], mu_r[:],
                                             start=True, stop=True)
                            nc.scalar.copy(mu_b[:, sl], bc1[:])
                            bc2 = p_psum_v.tile([128, 512], f32, tag="st")
                            nc.tensor.matmul(bc2[:], ones_row[:], r2_r[:],
                                             start=True, stop=True)
                            nc.scalar.copy(r2_b[:, sl], bc2[:])

                    if debug and h == 0:
                        nc.sync.dma_start(tens['dbg_ykv'][:, :], ykv[:])
                        nc.sync.dma_start(tens['dbg_mu'][:, :],
                                          mu_b[0:1, :])
                        nc.sync.dma_start(tens['dbg_r2'][:, :],
                                          r2_b[0:1, :])

                # ========================= Z / GATING =======================
                if h == HPC - 1:
                    p_xy1 = ctx.enter_context(
                        tc.tile_pool(name="xy1", bufs=1))
                    xy1_sb = p_xy1.tile([128, NT * T], bf16, tag="xy1")
                with ExitStack() as zctx:
                    p_wv = zctx.enter_context(
                        tc.tile_pool(name=f"wv{h}", bufs=1))
                    p_zs = zctx.enter_context(
                        tc.tile_pool(name=f"zs{h}", bufs=3))
                    p_zq = zctx.enter_context(
                        tc.tile_pool(name=f"zq{h}", bufs=2))
                    if h < HPC - 1:
                        p_xyw = zctx.enter_context(
                            tc.tile_pool(name=f"xyw{h}", bufs=2))

                    wv_sb = p_wv.tile([128, DT * N], bf16, tag="wv")
                    for dtt in range(DT):
                        nc.sync.dma_start(wv_sb[:, dtt * N:(dtt + 1) * N],
                                          wencvT[h, dtt * 128:(dtt + 1) * 128,
                                                 :])

                    for nt in range(NT):
                        if h < HPC - 1:
                            xyw = p_xyw.tile([128, T], bf16, tag="xyw")
                        else:
                            xyw = xy1_sb[:, nt * T:(nt + 1) * T]
                        # q = x_sparse^2 * r^2, full row of this n-tile
                        q_t = p_zq.tile([128, T], bf16, tag="q")
                        nc.vector.tensor_mul(
                            q_t[:], qsq[:, nt * T:(nt + 1) * T], r2_b[:])
                        for tb in range(TB):
                            ps3 = p_psum.tile([128, 512], f32, tag="mm")
                            for dtt in range(DT):
                                nc.tensor.matmul(
                                    ps3[:],
                                    wv_sb[:, dtt * N + nt * 128:
                                          dtt * N + nt * 128 + 128],
                                    ykv[:, dtt * T + tb * 512:
                                        dtt * T + tb * 512 + 512],
                                    start=(dtt == 0), stop=(dtt == DT - 1))
                            sl = slice(tb * 512, (tb + 1) * 512)
                            # v = Wsum[n]*mu[t] - z ; sqrelu(z-Wsum*mu)=min(v,0)*v
                            v_t = p_zs.tile([128, 512], f32, tag="v")
                            nc.vector.scalar_tensor_tensor(
                                v_t[:], mu_b[:, sl],
                                wsum_sb[:, h * NT + nt:h * NT + nt + 1],
                                ps3[:], op0=Alu.mult, op1=Alu.subtract)
                            g_t = p_zs.tile([128, 512], bf16, tag="g")
                            nc.vector.scalar_tensor_tensor(
                                g_t[:], v_t[:], 0.0, v_t[:],
                                op0=Alu.min, op1=Alu.mult)
                            nc.vector.tensor_mul(
                                xyw[:, sl], g_t[:], q_t[:, sl])
                        if h < HPC - 1:
                            nc.gpsimd.dma_start(xy_d[h, nt, :, :], xyw[:])

        # ============================ DECODER ============================
        CHUNK_TT = [4, 4, 4, 4]
        CHUNK_START = [0, 4, 8, 12]
        with ExitStack() as dctx:
            p_xyr = dctx.enter_context(tc.tile_pool(name="xyr", bufs=1))
            p_wd = dctx.enter_context(tc.tile_pool(name="wd", bufs=1))
            p_ym = dctx.enter_context(tc.tile_pool(name="ym", bufs=3))

            xy0_sb = p_xyr.tile([128, NT * T], bf16, tag="xyr0")
            wd_sb = p_wd.tile([128, HPC * NT * D], bf16, tag="wd")
            for ttg in range(4):
                for nt in range(NT):
                    eng = nc.sync if nt % 2 == 0 else nc.gpsimd
                    eng.dma_start(
                        xy0_sb[:, nt * T + ttg * 512:nt * T + (ttg + 1) * 512],
                        xy_d[0, nt, :, ttg * 512:(ttg + 1) * 512])
                if ttg == 0:
                    for db in range(DB):
                        for r in range(HPC * NT):
                            eng = nc.sync if r % 2 == 1 else nc.gpsimd
                            eng.dma_start(
                                wd_sb[:, r * D + db * 512:
                                      r * D + db * 512 + 512],
                                wdecT[r * 128:(r + 1) * 128,
                                      db * 512:(db + 1) * 512])
            xy_sb = [xy0_sb, xy1_sb]
            for chunk in range(len(CHUNK_TT)):
                for tt in range(CHUNK_START[chunk],
                                CHUNK_START[chunk] + CHUNK_TT[chunk]):
                    for db in range(DB):
                        ps4 = p_psum.tile([128, 512], f32, tag="mm")
                        idx = 0
                        for h in range(HPC):
                            for nt in range(NT):
                                nc.tensor.matmul(
                                    ps4[:],
                                    xy_sb[h][:, nt * T + tt * 128:
                                             nt * T + tt * 128 + 128],
                                    wd_sb[:, (h * NT + nt) * D + db * 512:
                                          (h * NT + nt) * D + db * 512 + 512],
                                    start=(idx == 0),
                                    stop=(idx == HPC * NT - 1))
                                idx += 1
                        ym_t = p_ym.tile([128, 512], f32, tag="ym")
                        nc.scalar.copy(ym_t[:], ps4[:])
                        nc.sync.dma_start(
                            bounce_in[tt * 128:(tt + 1) * 128,
                                      db * 512:(db + 1) * 512], ym_t[:])
                # reduce-scatter this chunk's rows; core c receives the
                # c-th 1/8 of the chunk's row range
                r0 = CHUNK_START[chunk] * 128
                rows = CHUNK_TT[chunk] * 128
                o0 = r0 // NCORES
                nc.gpsimd.collective_compute(
                    "ReduceScatter", Alu.add,
                    replica_groups=[list(range(NCORES))],
                    ins=[bounce_in[r0:r0 + rows, :].opt()],
                    outs=[bounce_out[o0:o0 + rows // NCORES, :].opt()])

        with ExitStack() as fctx:
            p_fin = fctx.enter_context(tc.tile_pool(name="fin", bufs=2))
            pieces = []
            for i in range(len(CHUNK_TT)):
                rows = CHUNK_TT[i] * 128 // NCORES
                base = CHUNK_START[i] * 128 // NCORES
                off = 0
                while off < rows:
                    p_ = min(128, rows - off)
                    pieces.append((base + off, p_))
                    off += p_
            for PO, P in pieces:
                yt = p_fin.tile([P, D], f32, tag="yt")
                nc.sync.dma_start(yt[:], bounce_out[PO:PO + P, :])
                xt = p_fin.tile([P, D], f32, tag="xt")
                nc.sync.dma_start(xt[:], xs_f32[PO:PO + P, :])

                mu_c = p_fin.tile([P, 1], f32, tag="mu_c")
                nc.vector.tensor_reduce(mu_c[:], yt[:],
                                        mybir.AxisListType.X, Alu.add)
                nc.scalar.mul(mu_c[:], mu_c[:], 1.0 / D)
                sq_t = p_fin.tile([P, D], f32, tag="sq_t")
                ssq_c = p_fin.tile([P, 1], f32, tag="ssq_c")
                nc.vector.tensor_mul(sq_t[:], yt[:], yt[:])
                nc.vector.tensor_reduce(ssq_c[:], sq_t[:],
                                        mybir.AxisListType.X, Alu.add)
                nc.scalar.mul(ssq_c[:], ssq_c[:], 1.0 / D)
                musq_c = p_fin.tile([P, 1], f32, tag="musq_c")
                nc.vector.tensor_mul(musq_c[:], mu_c[:], mu_c[:])
                nc.vector.tensor_sub(ssq_c[:], ssq_c[:], musq_c[:])
                nc.vector.tensor_scalar_add(ssq_c[:], ssq_c[:], LN_EPS)
                r_c = p_fin.tile([P, 1], f32, tag="r_c")
                nc.vector.reciprocal(r_c[:], ssq_c[:])
                nc.scalar.sqrt(r_c[:], r_c[:])

                zt = p_fin.tile([P, D], f32, tag="zt")
                nc.vector.tensor_scalar(zt[:], yt[:], mu_c[:], r_c[:],
                                        op0=Alu.subtract, op1=Alu.mult)
                nc.vector.tensor_add(zt[:], zt[:], xt[:])

                nc.vector.tensor_mul(sq_t[:], zt[:], zt[:])
                rr_c = p_fin.tile([P, 1], f32, tag="rr_c")
                nc.vector.tensor_reduce(rr_c[:], sq_t[:],
                                        mybir.AxisListType.X, Alu.add)
                nc.scalar.mul(rr_c[:], rr_c[:], 1.0 / D)
                nc.vector.tensor_scalar_add(rr_c[:], rr_c[:], RMS_EPS)
                nc.vector.reciprocal(rr_c[:], rr_c[:])
                nc.scalar.sqrt(rr_c[:], rr_c[:])

                ot = p_fin.tile([P, D], f32, tag="ot")
                nc.vector.tensor_scalar_mul(ot[:], zt[:], rr_c[:])
                nc.sync.dma_start(out_d[PO:PO + P, :], ot[:])


def build(debug=False):
    nc = bacc.Bacc("TRN2", target_bir_lowering=False, debug=False,
                   num_devices=NCORES)
    f32, bf16 = dt.float32, dt.bfloat16
    tens = {
        'x_bf': nc.dram_tensor("x_bf", [T, D], bf16, kind="ExternalInput"),
        'xT_bf': nc.dram_tensor("xT_bf", [D, T], bf16, kind="ExternalInput"),
        'xs_f32': nc.dram_tensor("xs_f32", [TS, D], f32,
                                 kind="ExternalInput"),
        'wencT': nc.dram_tensor("wencT", [HPC, D, N], bf16,
                                kind="ExternalInput"),
        'wencvT': nc.dram_tensor("wencvT", [HPC, D, N], bf16,
                                 kind="ExternalInput"),
        'wdecT': nc.dram_tensor("wdecT", [HPC * N, D], bf16,
                                kind="ExternalInput"),
        'wsumT': nc.dram_tensor("wsumT", [HPC, 128, NT], f32,
                                kind="ExternalInput"),
        'cosT': nc.dram_tensor("cosT", [128, T], bf16, kind="ExternalInput"),
        'sinT': nc.dram_tensor("sinT", [128, T], bf16, kind="ExternalInput"),
        'masks': nc.dram_tensor("masks", [4, 128, 512], f32,
                                kind="ExternalInput"),
        'out': nc.dram_tensor("out", [TS, D], f32, kind="ExternalOutput"),
        'xy_d': nc.dram_tensor("xy_d", [1, NT, 128, T], bf16,
                               kind="ExternalOutput" if debug
                               else "Internal"),
        'bounce_in': nc.dram_tensor(
            "bounce_in", [T, D], f32,
            kind="ExternalOutput" if debug else "Internal"),
        'bounce_out': nc.dram_tensor(
            "bounce_out", [TS, D], f32,
            kind="ExternalOutput" if debug else "Internal"),
    }
    if debug:
        tens['dbg_qtr'] = nc.dram_tensor("dbg_qtr", [128, NT * T], bf16,
                                         kind="ExternalOutput")
        tens['dbg_ykv'] = nc.dram_tensor("dbg_ykv", [128, DT * T], bf16,
                                         kind="ExternalOutput")
        tens['dbg_mu'] = nc.dram_tensor("dbg_mu", [1, T], f32,
                                        kind="ExternalOutput")
        tens['dbg_r2'] = nc.dram_tensor("dbg_r2", [1, T], bf16,
                                        kind="ExternalOutput")

    with tile.TileContext(nc) as tc:
        _emit(nc, tc, tens, debug=debug)
    nc.compile()
    return nc


def make_in_maps(x, W_enc, W_enc_v, W_dec):
    x2 = np.asarray(x, FP32).reshape(T, D)
    x_bf = x2.astype(BF)
    xT_bf = np.ascontiguousarray(x2.T).astype(BF)
    cosT, sinT = _rope_tables()
    cosT, sinT = cosT.astype(BF), sinT.astype(BF)
    masks = _masks()
    wsum = np.asarray(W_enc_v, FP32).sum(axis=2)          # (NH, N)

    in_maps = []
    for k in range(NCORES):
        h0 = HPC * k
        wencT = np.ascontiguousarray(
            np.asarray(W_enc[h0:h0 + HPC], FP32).transpose(0, 2, 1)
        ).astype(BF)
        wencvT = np.ascontiguousarray(
            np.asarray(W_enc_v[h0:h0 + HPC], FP32).transpose(0, 2, 1)
        ).astype(BF)
        wdecT = np.ascontiguousarray(
            np.asarray(W_dec[:, h0 * N:(h0 + HPC) * N], FP32).T
        ).astype(BF)
        wsumT = np.ascontiguousarray(
            wsum[h0:h0 + HPC].reshape(HPC, NT, 128).transpose(0, 2, 1))
        in_maps.append({
            'x_bf': x_bf,
            'xT_bf': xT_bf,
            'xs_f32': np.ascontiguousarray(np.concatenate(
                [x2[cs * 128 + ct * 16 * k:cs * 128 + ct * 16 * k + ct * 16]
                 for cs, ct in zip((0, 4, 8, 12), (4, 4, 4, 4))], axis=0)),
            'wencT': wencT,
            'wencvT': wencvT,
            'wdecT': wdecT,
            'wsumT': wsumT,
            'cosT': cosT,
            'sinT': sinT,
            'masks': masks,
        })
    return in_maps


_nc_cache = {}


def get_nc(debug=False):
    if debug not in _nc_cache:
        _nc_cache[debug] = build(debug=debug)
    return _nc_cache[debug]


def run(x, W_enc, W_enc_v, W_dec, debug=False, trace=False):
    nc = get_nc(debug=debug)
    in_maps = make_in_maps(x, W_enc, W_enc_v, W_dec)
    res = bass_utils.run_bass_kernel_spmd(
        nc, in_maps, core_ids=list(range(NCORES)), trace=trace)
    # chunked reduce-scatter: core c's piece i holds the c-th 1/8 of
    # chunk i's row range
    out = np.empty((T, D), np.float32)
    for c in range(NCORES):
        oc = res.results[c]['out']
        o = 0
        for cs, ct in zip((0, 4, 8, 12), (4, 4, 4, 4)):
            n = ct * 16
            g = cs * 128 + n * c
            out[g:g + n] = oc[o:o + n]
            o += n
    return out.reshape(B, T, D), res


def kernel(x, W_enc, W_enc_v, W_dec):
    out, _ = run(x, W_enc, W_enc_v, W_dec)
    return out.astype(np.float32)


# revision 34
# speedup vs baseline: 1.0141x; 1.0141x over previous
"""Trainium2 Bass kernel for nn_BDHLayer (sparse attention / BDH layer).

Sharding: 16 heads across 8 cores (2 heads per core, tensor parallel).
Decoder partial sums are combined with an on-chip ReduceScatter; each core
then applies the final layernorm+residual+rmsnorm to its T/8 slice.

All matmuls run in bf16 (fp32 PSUM accumulation). Host pre-transposes
weights/activations so every contraction dim lands on SBUF partitions.
The middle layernorm is applied as a post-GEMM correction:
  sqrelu(LN(yKV) @ Wv^T) = min(v,0)*v*r^2 with v = Wsum*mu - z,
  z = yKV @ Wv^T, Wsum = sum_d Wv, r^2 = 1/(var+eps).
"""

import sys

sys.path.insert(0, '/opt/trn_rl_repo')

import numpy as np
import ml_dtypes

import concourse.bass as bass
import concourse.bacc as bacc
import concourse.mybir as mybir
from concourse import tile
from concourse import bass_utils

BF = ml_dtypes.bfloat16
FP32 = np.float32

B, T, D = 1, 2048, 1024
NH = 16
N = 1024            # neurons per head
CS = 256            # rotary chunk size
BASE = 2.0 ** 16
SCALE_BASE = 512.0
LN_EPS = 1e-5
RMS_EPS = 1e-6

NCORES = 8
HPC = NH // NCORES  # heads per core = 2
TS = T // NCORES    # output rows per core = 256

NT = N // 128       # 8 n-tiles per head
DT = D // 128       # 8 d-tiles
TT = T // 128       # 16 t-tiles
TB = T // 512       # 4 t-blocks
DB = D // 512       # 2 d-blocks

dt = mybir.dt
Alu = mybir.AluOpType
Act = mybir.ActivationFunctionType


# ---------------------------------------------------------------- host tables

def _rope_tables():
    idx = np.arange(0, CS, 2, dtype=np.float64)
    inv_freq = 1.0 / (BASE ** (idx / CS))
    t = np.arange(T, dtype=np.float64)
    freqs = t[:, None] * inv_freq[None, :]              # (T, 128)
    scale_vec = (idx + 0.4 * CS) / (1.4 * CS)
    power = (t - T // 2) / SCALE_BASE
    scale = scale_vec[None, :] ** power[:, None]        # (T, 128)
    cos = (np.cos(freqs) * scale).astype(np.float32)
    sin = (np.sin(freqs) * scale).astype(np.float32)
    # transpose to (128, T): row = pair index within chunk, col = t
    return np.ascontiguousarray(cos.T), np.ascontiguousarray(sin.T)


def _masks():
    # scoresT tile layout: [u_p (128), t_f (512)]; diagonal block j keeps
    # strictly-causal u < t, i.e. 128*j + u_p < t_f.
    m = np.zeros((4, 128, 512), dtype=np.float32)
    up = np.arange(128)[:, None]
    tf = np.arange(512)[None, :]
    for j in range(4):
        m[j] = (128 * j + up < tf).astype(np.float32)
    return m


# ------------------------------------------------------------------- builder

def _emit(nc, tc, tens, debug=False):
    x_bf, xT_bf, xs_f32 = tens['x_bf'], tens['xT_bf'], tens['xs_f32']
    wencT, wencvT, wdecT = tens['wencT'], tens['wencvT'], tens['wdecT']
    wsumT, cosT_d, sinT_d, masks_d = (tens['wsumT'], tens['cosT'],
                                      tens['sinT'], tens['masks'])
    out_d, xy_d = tens['out'], tens['xy_d']
    bounce_in, bounce_out = tens['bounce_in'], tens['bounce_out']

    f32, bf16 = dt.float32, dt.bfloat16

    from contextlib import ExitStack
    with ExitStack() as ctx:
        p_const = ctx.enter_context(
            tc.tile_pool(name="const", bufs=1, side="right"))
        p_x = ctx.enter_context(
            tc.tile_pool(name="xnat", bufs=1, side="right"))
        p_psum = ctx.enter_context(
            tc.tile_pool(name="psum", bufs=6, space="PSUM"))
        p_psum_v = ctx.enter_context(
            tc.tile_pool(name="psumv", bufs=1, space="PSUM"))

        # ---- constants
        cos_sb = p_const.tile([128, T], bf16, tag="cos")
        sin_sb = p_const.tile([128, T], bf16, tag="sin")
        nc.gpsimd.dma_start(cos_sb[:], cosT_d[:])
        nc.gpsimd.dma_start(sin_sb[:], sinT_d[:])
        mask_sb = p_const.tile([128, 4 * 512], f32, tag="masks")
        wsum_sb = p_const.tile([128, HPC * NT], f32, tag="wsum")
        for h in range(HPC):
            nc.sync.dma_start(wsum_sb[:, h * NT:(h + 1) * NT],
                              wsumT[h, :, :])
        ones_bf = p_const.tile([128, 1], bf16, tag="ones_bf")
        nc.vector.memset(ones_bf[:], 1.0)
        ones_row = p_const.tile([1, 128], bf16, tag="ones_row")
        nc.vector.memset(ones_row[:], 1.0)

        # ---- x natural layout (persistent): tile tt holds x[128tt:+128, :]
        x_sb = p_x.tile([128, TT * D], bf16, tag="x")
        bulk_loaded = [False]

        def load_bulk():
            if bulk_loaded[0]:
                return
            bulk_loaded[0] = True
            for j in range(4):
                nc.gpsimd.dma_start(mask_sb[:, j * 512:(j + 1) * 512],
                                    masks_d[j, :, :])
            for tt in range(TT):
                nc.gpsimd.dma_start(x_sb[:, tt * D:(tt + 1) * D],
                                    x_bf[tt * 128:(tt + 1) * 128, :])

        for h in range(HPC):
            with ExitStack() as hctx:
                # x_sparse^2 (pre-rope), full head — lives enc -> gating
                p_qsq = hctx.enter_context(
                    tc.tile_pool(name=f"qsq{h}", bufs=1, side="right"))
                qsq = p_qsq.tile([128, NT * T], bf16, tag="qsq")

                # =========================== ENC + ROPE ====================
                with ExitStack() as ectx:
                    p_qtr = ectx.enter_context(
                        tc.tile_pool(name=f"qtr{h}", bufs=1))
                    qtr = p_qtr.tile([128, NT * T], bf16, tag="qtr")

                    with ExitStack() as e2:
                        p_enc = e2.enter_context(
                            tc.tile_pool(name=f"enc{h}", bufs=1))
                        p_xts = e2.enter_context(
                            tc.tile_pool(name=f"xts{h}", bufs=1))
                        p_scr = e2.enter_context(
                            tc.tile_pool(name=f"escr{h}", bufs=3))

                        wenc_sb = p_enc.tile([128, DT * N], bf16, tag="wenc")
                        for dtt in range(DT):
                            eng = nc.sync if dtt % 2 == 0 else nc.gpsimd
                            eng.dma_start(
                                wenc_sb[:, dtt * N:(dtt + 1) * N],
                                wencT[h, dtt * 128:(dtt + 1) * 128, :])

                        # full xT resident; slices arrive per (tb, dtt)
                        xfull = p_xts.tile([128, TB * DT * 512], bf16,
                                           tag="xts")
                        for tb in range(TB):
                            for dtt in range(DT):
                                eng = nc.sync if dtt % 2 == 1 else nc.gpsimd
                                eng.dma_start(
                                    xfull[:, tb * DT * 512 + dtt * 512:
                                          tb * DT * 512 + (dtt + 1) * 512],
                                    xT_bf[dtt * 128:(dtt + 1) * 128,
                                          tb * 512:(tb + 1) * 512])
                        p_rt = e2.enter_context(
                            tc.tile_pool(name=f"rt{h}", bufs=3))
                        for nt in range(NT):
                            for tb in range(TB):
                                ps = p_psum.tile([128, 512], f32, tag="mm")
                                for dtt in range(DT):
                                    nc.tensor.matmul(
                                        ps[:],
                                        wenc_sb[:, dtt * N + nt * 128:
                                                dtt * N + nt * 128 + 128],
                                        xfull[:, tb * DT * 512 + dtt * 512:
                                              tb * DT * 512 + (dtt + 1) * 512],
                                        start=(dtt == 0), stop=(dtt == DT - 1))
                                relu_t = p_scr.tile([128, 512], f32,
                                                    tag="relu")
                                nc.scalar.activation(relu_t[:], ps[:],
                                                     Act.Relu)
                                nc.vector.tensor_mul(
                                    qsq[:, nt * T + tb * 512:
                                        nt * T + tb * 512 + 512],
                                    relu_t[:], relu_t[:])
                            if nt % 2 == 1:
                                # rope chunk (nt-1, nt) as soon as pair done
                                c = nt // 2
                                a = qsq[:, (2 * c) * T:(2 * c + 1) * T]
                                b = qsq[:, (2 * c + 1) * T:(2 * c + 2) * T]
                                t1 = p_rt.tile([128, T], bf16, tag="rtmp")
                                t2 = p_rt.tile([128, T], bf16, tag="rtmp")
                                nc.vector.tensor_mul(t1[:], a, cos_sb[:])
                                nc.vector.tensor_mul(t2[:], b, sin_sb[:])
                                nc.vector.tensor_sub(
                                    qtr[:, (2 * c) * T:(2 * c + 1) * T],
                                    t1[:], t2[:])
                                t3 = p_rt.tile([128, T], bf16, tag="rtmp")
                                t4 = p_rt.tile([128, T], bf16, tag="rtmp")
                                nc.vector.tensor_mul(t3[:], b, cos_sb[:])
                                nc.vector.tensor_mul(t4[:], a, sin_sb[:])
                                nc.vector.tensor_add(
                                    qtr[:, (2 * c + 1) * T:(2 * c + 2) * T],
                                    t3[:], t4[:])

                    load_bulk()
                    if debug and h == 0:
                        nc.sync.dma_start(tens['dbg_qtr'][:, :], qtr[:])

                    # ======================= SCORES + yKV ===================
                    p_ykv = hctx.enter_context(
                        tc.tile_pool(name=f"ykv{h}", bufs=1, side="right"))
                    p_vec = hctx.enter_context(
                        tc.tile_pool(name=f"vec{h}", bufs=1, side="right"))
                    p_vrow = hctx.enter_context(
                        tc.tile_pool(name=f"vrow{h}", bufs=1, side="right"))
                    ykv = p_ykv.tile([128, DT * T], bf16, tag="ykv")
                    # stat rows broadcast across partitions (via K=1 matmul)
                    mu_b = p_vec.tile([128, T], f32, tag="mu_b")
                    r2_b = p_vec.tile([128, T], bf16, tag="r2_b")

                    with ExitStack() as sctx:
                        p_sct = sctx.enter_context(
                            tc.tile_pool(name=f"sct{h}", bufs=1))
                        p_sq = sctx.enter_context(
                            tc.tile_pool(name=f"sq{h}", bufs=1))

                        for tb in range(TB):
                            ub_max = 4 * tb + 4
                            sct = p_sct.tile([128, 16 * 512], bf16, tag="sct")
                            for ub in range(ub_max):
                                j = ub - 4 * tb
                                off = 128 * j if j > 0 else 0
                                w = 512 - off
                                ps = p_psum.tile([128, 512], f32, tag="mm")
                                for nt in range(NT):
                                    nc.tensor.matmul(
                                        ps[:, :w],
                                        qtr[:, nt * T + ub * 128:
                                            nt * T + ub * 128 + 128],
                                        qtr[:, nt * T + tb * 512 + off:
                                            nt * T + (tb + 1) * 512],
                                        start=(nt == 0), stop=(nt == NT - 1))
                                base = ub * 512
                                if j >= 0:
                                    if off:
                                        nc.vector.memset(
                                            sct[:, base:base + off], 0.0)
                                    nc.vector.tensor_mul(
                                        sct[:, base + off:base + 512],
                                        ps[:, :w],
                                        mask_sb[:, j * 512 + off:
                                                (j + 1) * 512])
                                else:
                                    nc.scalar.copy(sct[:, base:base + 512],
                                                   ps[:])

                            sq_full = p_sq.tile([128, DT * 512], bf16,
                                                tag="sq")
                            for dtt in range(DT):
                                ps2 = p_psum.tile([128, 512], f32, tag="mm")
                                for ub in range(ub_max):
                                    nc.tensor.matmul(
                                        ps2[:],
                                        x_sb[:, ub * D + dtt * 128:
                                             ub * D + dtt * 128 + 128],
                                        sct[:, ub * 512:(ub + 1) * 512],
                                        start=(ub == 0),
                                        stop=(ub == ub_max - 1))
                                nc.scalar.copy(
                                    ykv[:, dtt * T + tb * 512:
                                        dtt * T + tb * 512 + 512], ps2[:])
                                nc.scalar.square(
                                    sq_full[:, dtt * 512:(dtt + 1) * 512],
                                    ps2[:])

                            mean_ps = p_psum_v.tile([1, 512], f32, tag="st")
                            for dtt in range(DT):
                                nc.tensor.matmul(
                                    mean_ps[:], ones_bf[:],
                                    ykv[:, dtt * T + tb * 512:
                                        dtt * T + tb * 512 + 512],
                                    start=(dtt == 0), stop=(dtt == DT - 1))
                            ssq_ps = p_psum_v.tile([1, 512], f32, tag="st")
                            for dtt in range(DT):
                                nc.tensor.matmul(
                                    ssq_ps[:], ones_bf[:],
                                    sq_full[:, dtt * 512:(dtt + 1) * 512],
                                    start=(dtt == 0), stop=(dtt == DT - 1))
                            sl = slice(tb * 512, (tb + 1) * 512)
                            mu_r = p_vrow.tile([1, 512], bf16, tag="mu_r")
                            ssq_r = p_vrow.tile([1, 512], f32, tag="ssq_r")
                            musq_r = p_vrow.tile([1, 512], f32, tag="musq_r")
                            r2_r = p_vrow.tile([1, 512], bf16, tag="r2_r")
                            nc.scalar.mul(mu_r[:], mean_ps[:], 1.0 / D)
                            nc.scalar.mul(ssq_r[:], ssq_ps[:], 1.0 / D)
                            nc.vector.tensor_mul(musq_r[:], mu_r[:], mu_r[:])
                            nc.vector.tensor_sub(ssq_r[:], ssq_r[:],
                                                 musq_r[:])
                            nc.vector.tensor_scalar_add(
                                ssq_r[:], ssq_r[:], LN_EPS)
                            nc.vector.reciprocal(ssq_r[:], ssq_r[:])
                            nc.vector.tensor_copy(r2_r[:], ssq_r[:])
                            # broadcast mu/r2 rows to all 128 partitions
                            bc1 = p_psum_v.tile([128, 512], f32, tag="bc")
                            nc.tensor.matmul(bc1[:], ones_row[:], mu_r[:],
                                             start=True, stop=True)
                            nc.scalar.copy(mu_b[:, sl], bc1[:])
                            bc2 = p_psum_v.tile([128, 512], f32, tag="bc")
                            nc.tensor.matmul(bc2[:], ones_row[:], r2_r[:],
                                             start=True, stop=True)
                            nc.scalar.copy(r2_b[:, sl], bc2[:])

                    if debug and h == 0:
                        nc.sync.dma_start(tens['dbg_ykv'][:, :], ykv[:])
                        nc.sync.dma_start(tens['dbg_mu'][:, :],
                                          mu_b[0:1, :])
                        nc.sync.dma_start(tens['dbg_r2'][:, :],
                                          r2_b[0:1, :])

                # ========================= Z / GATING =======================
                if h == HPC - 1:
                    p_xy1 = ctx.enter_context(
                        tc.tile_pool(name="xy1", bufs=1))
                    xy1_sb = p_xy1.tile([128, NT * T], bf16, tag="xy1")
                with ExitStack() as zctx:
                    p_wv = zctx.enter_context(
                        tc.tile_pool(name=f"wv{h}", bufs=1))
                    p_zs = zctx.enter_context(
                        tc.tile_pool(name=f"zs{h}", bufs=3))
                    p_zq = zctx.enter_context(
                        tc.tile_pool(name=f"zq{h}", bufs=2))
                    if h < HPC - 1:
                        p_xyw = zctx.enter_context(
                            tc.tile_pool(name=f"xyw{h}", bufs=2))

                    wv_sb = p_wv.tile([128, DT * N], bf16, tag="wv")
                    for dtt in range(DT):
                        nc.sync.dma_start(wv_sb[:, dtt * N:(dtt + 1) * N],
                                          wencvT[h, dtt * 128:(dtt + 1) * 128,
                                                 :])

                    for nt in range(NT):
                        if h < HPC - 1:
                            xyw = p_xyw.tile([128, T], bf16, tag="xyw")
                        else:
                            xyw = xy1_sb[:, nt * T:(nt + 1) * T]
                        # q = x_sparse^2 * r^2, full row of this n-tile
                        q_t = p_zq.tile([128, T], bf16, tag="q")
                        nc.vector.tensor_mul(
                            q_t[:], qsq[:, nt * T:(nt + 1) * T], r2_b[:])
                        for tb in range(TB):
                            ps3 = p_psum.tile([128, 512], f32, tag="mm")
                            for dtt in range(DT):
                                nc.tensor.matmul(
                                    ps3[:],
                                    wv_sb[:, dtt * N + nt * 128:
                                          dtt * N + nt * 128 + 128],
                                    ykv[:, dtt * T + tb * 512:
                                        dtt * T + tb * 512 + 512],
                                    start=(dtt == 0), stop=(dtt == DT - 1))
                            sl = slice(tb * 512, (tb + 1) * 512)
                            # v = Wsum[n]*mu[t] - z ; sqrelu(z-Wsum*mu)=min(v,0)*v
                            v_t = p_zs.tile([128, 512], f32, tag="v")
                            nc.vector.scalar_tensor_tensor(
                                v_t[:], mu_b[:, sl],
                                wsum_sb[:, h * NT + nt:h * NT + nt + 1],
                                ps3[:], op0=Alu.mult, op1=Alu.subtract)
                            g_t = p_zs.tile([128, 512], bf16, tag="g")
                            nc.vector.scalar_tensor_tensor(
                                g_t[:], v_t[:], 0.0, v_t[:],
                                op0=Alu.min, op1=Alu.mult)
                            nc.vector.tensor_mul(
                                xyw[:, sl], g_t[:], q_t[:, sl])
                        if h < HPC - 1:
                            nc.gpsimd.dma_start(xy_d[h, nt, :, :], xyw[:])

        # ============================ DECODER ============================
        CHUNK_TT = [4, 4, 4, 4]
        CHUNK_START = [0, 4, 8, 12]
        with ExitStack() as dctx:
            p_xyr = dctx.enter_context(tc.tile_pool(name="xyr", bufs=1))
            p_wd = dctx.enter_context(tc.tile_pool(name="wd", bufs=1))
            p_ym = dctx.enter_context(tc.tile_pool(name="ym", bufs=3))

            xy0_sb = p_xyr.tile([128, NT * T], bf16, tag="xyr0")
            wd_sb = p_wd.tile([128, HPC * NT * D], bf16, tag="wd")
            for ttg in range(4):
                for nt in range(NT):
                    eng = nc.sync if nt % 2 == 0 else nc.gpsimd
                    eng.dma_start(
                        xy0_sb[:, nt * T + ttg * 512:nt * T + (ttg + 1) * 512],
                        xy_d[0, nt, :, ttg * 512:(ttg + 1) * 512])
                if ttg == 0:
                    for db in range(DB):
                        for r in range(HPC * NT):
                            eng = nc.sync if r % 2 == 1 else nc.gpsimd
                            eng.dma_start(
                                wd_sb[:, r * D + db * 512:
                                      r * D + db * 512 + 512],
                                wdecT[r * 128:(r + 1) * 128,
                                      db * 512:(db + 1) * 512])
            xy_sb = [xy0_sb, xy1_sb]
            for chunk in range(len(CHUNK_TT)):
                for tt in range(CHUNK_START[chunk],
                                CHUNK_START[chunk] + CHUNK_TT[chunk]):
                    for db in range(DB):
                        ps4 = p_psum.tile([128, 512], f32, tag="mm")
                        idx = 0
                        for h in range(HPC):
                            for nt in range(NT):
                                nc.tensor.matmul(
                                    ps4[:],
                                    xy_sb[h][:, nt * T + tt * 128:
                                             nt * T + tt * 128 + 128],
                                    wd_sb[:, (h * NT + nt) * D + db * 512:
                                          (h * NT + nt) * D + db * 512 + 512],
                                    start=(idx == 0),
                                    stop=(idx == HPC * NT - 1))
                                idx += 1
                        ym_t = p_ym.tile([128, 512], f32, tag="ym")
                        nc.scalar.copy(ym_t[:], ps4[:])
                        nc.sync.dma_start(
                            bounce_in[tt * 128:(tt + 1) * 128,
                                      db * 512:(db + 1) * 512], ym_t[:])
                # reduce-scatter this chunk's rows; core c receives the
                # c-th 1/8 of the chunk's row range
                r0 = CHUNK_START[chunk] * 128
                rows = CHUNK_TT[chunk] * 128
                o0 = r0 // NCORES
                nc.gpsimd.collective_compute(
                    "ReduceScatter", Alu.add,
                    replica_groups=[list(range(NCORES))],
                    ins=[bounce_in[r0:r0 + rows, :].opt()],
                    outs=[bounce_out[o0:o0 + rows // NCORES, :].opt()])

        with ExitStack() as fctx:
            p_fin = fctx.enter_context(tc.tile_pool(name="fin", bufs=2))
            pieces = []
            for i in range(len(CHUNK_TT)):
                rows = CHUNK_TT[i] * 128 // NCORES
                base = CHUNK_START[i] * 128 // NCORES
                off = 0
                while off < rows:
                    p_ = min(128, rows - off)
                    pieces.append((base + off, p_))
                    off += p_
            for PO, P in pieces:
                yt = p_fin.tile([P, D], f32, tag="yt")
                nc.sync.dma_start(yt[:], bounce_out[PO:PO + P, :])
                xt = p_fin.tile([P, D], f32, tag="xt")
                nc.sync.dma_start(xt[:], xs_f32[PO:PO + P, :])

                mu_c = p_fin.tile([P, 1], f32, tag="mu_c")
                nc.vector.tensor_reduce(mu_c[:], yt[:],
                                        mybir.AxisListType.X, Alu.add)
                nc.scalar.mul(mu_c[:], mu_c[:], 1.0 / D)
                sq_t = p_fin.tile([P, D], f32, tag="sq_t")
                ssq_c = p_fin.tile([P, 1], f32, tag="ssq_c")
                nc.vector.tensor_mul(sq_t[:], yt[:], yt[:])
                nc.vector.tensor_reduce(ssq_c[:], sq_t[:],
                                        mybir.AxisListType.X, Alu.add)
                nc.scalar.mul(ssq_c[:], ssq_c[:], 1.0 / D)
                musq_c = p_fin.tile([P, 1], f32, tag="musq_c")
                nc.vector.tensor_mul(musq_c[:], mu_c[:], mu_c[:])
                nc.vector.tensor_sub(ssq_c[:], ssq_c[:], musq_c[:])
                nc.vector.tensor_scalar_add(ssq_c[:], ssq_c[:], LN_EPS)
                r_c = p_fin.tile([P, 1], f32, tag="r_c")
                nc.vector.reciprocal(r_c[:], ssq_c[:])
                nc.scalar.sqrt(r_c[:], r_c[:])

                zt = p_fin.tile([P, D], f32, tag="zt")
                nc.vector.tensor_scalar(zt[:], yt[:], mu_c[:], r_c[:],
                                        op0=Alu.subtract, op1=Alu.mult)
                nc.vector.tensor_add(zt[:], zt[:], xt[:])

                nc.vector.tensor_mul(sq_t[:], zt[:], zt[:])
                rr_c = p_fin.tile([P, 1], f32, tag="rr_c")
                nc.vector.tensor_reduce(rr_c[:], sq_t[:],
                                        mybir.AxisListType.X, Alu.add)
                nc.scalar.mul(rr_c[:], rr_c[:], 1.0 / D)
                nc.vector.tensor_scalar_add(rr_c[:], rr_c[:], RMS_EPS)
                nc.vector.reciprocal(rr_c[:], rr_c[:])
                nc.scalar.sqrt(rr_c[:], rr_c[:])

                ot = p_fin.tile([P, D], f32, tag="ot")
                nc.vector.tensor_scalar_mul(ot[:], zt[:], rr_c[:])
                nc.sync.dma_start(out_d[PO:PO + P, :], ot[:])


def build(debug=False):
    nc = bacc.Bacc("TRN2", target_bir_lowering=False, debug=False,
                   num_devices=NCORES)
    f32, bf16 = dt.float32, dt.bfloat16
    tens = {
        'x_bf': nc.dram_tensor("x_bf", [T, D], bf16, kind="ExternalInput"),
        'xT_bf': nc.dram_tensor("xT_bf", [D, T], bf16, kind="ExternalInput"),
        'xs_f32': nc.dram_tensor("xs_f32", [TS, D], f32,
                                 kind="ExternalInput"),
        'wencT': nc.dram_tensor("wencT", [HPC, D, N], bf16,
                                kind="ExternalInput"),
        'wencvT': nc.dram_tensor("wencvT", [HPC, D, N], bf16,
                                 kind="ExternalInput"),
        'wdecT': nc.dram_tensor("wdecT", [HPC * N, D], bf16,
                                kind="ExternalInput"),
        'wsumT': nc.dram_tensor("wsumT", [HPC, 128, NT], f32,
                                kind="ExternalInput"),
        'cosT': nc.dram_tensor("cosT", [128, T], bf16, kind="ExternalInput"),
        'sinT': nc.dram_tensor("sinT", [128, T], bf16, kind="ExternalInput"),
        'masks': nc.dram_tensor("masks", [4, 128, 512], f32,
                                kind="ExternalInput"),
        'out': nc.dram_tensor("out", [TS, D], f32, kind="ExternalOutput"),
        'xy_d': nc.dram_tensor("xy_d", [1, NT, 128, T], bf16,
                               kind="ExternalOutput" if debug
                               else "Internal"),
        'bounce_in': nc.dram_tensor(
            "bounce_in", [T, D], f32,
            kind="ExternalOutput" if debug else "Internal"),
        'bounce_out': nc.dram_tensor(
            "bounce_out", [TS, D], f32,
            kind="ExternalOutput" if debug else "Internal"),
    }
    if debug:
        tens['dbg_qtr'] = nc.dram_tensor("dbg_qtr", [128, NT * T], bf16,
                                         kind="ExternalOutput")
        tens['dbg_ykv'] = nc.dram_tensor("dbg_ykv", [128, DT * T], bf16,
                                         kind="ExternalOutput")
        tens['dbg_mu'] = nc.dram_tensor("dbg_mu", [1, T], f32,
                                        kind="ExternalOutput")
        tens['dbg_r2'] = nc.dram_tensor("dbg_r2", [1, T], bf16,
                                        kind="ExternalOutput")

    with tile.TileContext(nc) as tc:
        _emit(nc, tc, tens, debug=debug)
    nc.compile()
    return nc


def make_in_maps(x, W_enc, W_enc_v, W_dec):
    x2 = np.asarray(x, FP32).reshape(T, D)
    x_bf = x2.astype(BF)
    xT_bf = np.ascontiguousarray(x2.T).astype(BF)
    cosT, sinT = _rope_tables()
    cosT, sinT = cosT.astype(BF), sinT.astype(BF)
    masks = _masks()
    wsum = np.asarray(W_enc_v, FP32).sum(axis=2)          # (NH, N)

    in_maps = []
    for k in range(NCORES):
        h0 = HPC * k
        wencT = np.ascontiguousarray(
            np.asarray(W_enc[h0:h0 + HPC], FP32).transpose(0, 2, 1)
        ).astype(BF)
        wencvT = np.ascontiguousarray(
            np.asarray(W_enc_v[h0:h0 + HPC], FP32).transpose(0, 2, 1)
        ).astype(BF)
        wdecT = np.ascontiguousarray(
            np.asarray(W_dec[:, h0 * N:(h0 + HPC) * N], FP32).T
        ).astype(BF)
        wsumT = np.ascontiguousarray(
            wsum[h0:h0 + HPC].reshape(HPC, NT, 128).transpose(0, 2, 1))
        in_maps.append({
            'x_bf': x_bf,
            'xT_bf': xT_bf,
            'xs_f32': np.ascontiguousarray(np.concatenate(
                [x2[cs * 128 + ct * 16 * k:cs * 128 + ct * 16 * k + ct * 16]
                 for cs, ct in zip((0, 4, 8, 12), (4, 4, 4, 4))], axis=0)),
            'wencT': wencT,
            'wencvT': wencvT,
            'wdecT': wdecT,
            'wsumT': wsumT,
            'cosT': cosT,
            'sinT': sinT,
            'masks': masks,
        })
    return in_maps


_nc_cache = {}


def get_nc(debug=False):
    if debug not in _nc_cache:
        _nc_cache[debug] = build(debug=debug)
    return _nc_cache[debug]


def run(x, W_enc, W_enc_v, W_dec, debug=False, trace=False):
    nc = get_nc(debug=debug)
    in_maps = make_in_maps(x, W_enc, W_enc_v, W_dec)
    res = bass_utils.run_bass_kernel_spmd(
        nc, in_maps, core_ids=list(range(NCORES)), trace=trace)
    # chunked reduce-scatter: core c's piece i holds the c-th 1/8 of
    # chunk i's row range
    out = np.empty((T, D), np.float32)
    for c in range(NCORES):
        oc = res.results[c]['out']
        o = 0
        for cs, ct in zip((0, 4, 8, 12), (4, 4, 4, 4)):
            n = ct * 16
            g = cs * 128 + n * c
            out[g:g + n] = oc[o:o + n]
            o += n
    return out.reshape(B, T, D), res


def kernel(x, W_enc, W_enc_v, W_dec):
    out, _ = run(x, W_enc, W_enc_v, W_dec)
    return out.astype(np.float32)
